# revision 8
# baseline (speedup 1.0000x reference)
"""Trainium2 Bass kernel for the 2-layer GAT message-passing network.

Contract: kernel(**inputs) takes FULL inputs (as from setup_inputs()) and
returns the FULL [8192, 64] float32 output. Inside, work is sharded row-wise
across 8 NeuronCores (1024 attention rows each); the [N,F] projections are
gathered with on-device AllGather collectives between layers.

Self-contained: only imports concourse (the Bass stack) + numpy/jax/ml_dtypes.
"""
import numpy as np
import ml_dtypes

import concourse.bass as bass
import concourse.bacc as bacc
import concourse.mybir as mybir
from concourse import bass_utils
from concourse.tile import TileContext
from contextlib import ExitStack

FP32 = mybir.dt.float32
BF16 = mybir.dt.bfloat16
FP16 = mybir.dt.float16
AF = mybir.ActivationFunctionType
ALU = mybir.AluOpType

N, D, H, O = 8192, 256, 128, 64
NCORES = 8
RL = N // NCORES          # rows per core (1024)
MASK_BIG = 500.0
SB = 512                  # attention r-superblock

_CACHED = {}


def _gat_attention(
    nc, tc, ctx, *, name, adjbf, wh_sb, ones_sb, i500_bf, i1_f16, id_bf,
    one1_f32, neg500, sdst_arr, ssrc_arr, out_cb, R_LOCAL, n, F,
    relu_act_frac=0.15,
):
    """One GAT attention layer (attention only; Wh/scores precomputed).

    [j, r] tiles: psum = 0.2z (K=2 fp16 mm) ; t = relu(0.8z) via DVE/ACT ;
    PE folds += I@t and += 500I@maskT ; em = Exp(psum - 500) ;
    psum_hT[F,r] += Wh_jb.T @ em ; den += ones.T @ em ; epilogue transposes
    back and normalizes.
    """
    n_jb = n // 128
    n_sb = R_LOCAL // SB

    zpool = ctx.enter_context(tc.tile_pool(name=f"{name}_z", bufs=3, space="PSUM"))
    opool = ctx.enter_context(tc.tile_pool(name=f"{name}_o", bufs=1, space="PSUM"))
    eppool = ctx.enter_context(tc.tile_pool(name=f"{name}_ep", bufs=2, space="PSUM"))
    spool = ctx.enter_context(tc.tile_pool(name=f"{name}_s", bufs=3))
    mpool = ctx.enter_context(tc.tile_pool(name=f"{name}_m", bufs=4))
    fpool = ctx.enter_context(tc.tile_pool(name=f"{name}_f", bufs=2))

    for sb in range(n_sb):
        r0 = sb * SB
        psum_hT = opool.tile([128, SB], FP32, tag="hT")
        psum_den = opool.tile([1, SB], FP32, tag="den")
        for jb in range(n_jb):
            maskT = mpool.tile([128, SB], BF16, tag="mask")
            dma_eng = nc.sync if jb % 2 == 0 else nc.scalar
            dma_eng.dma_start(
                maskT[:], adjbf[r0:r0 + SB, jb * 128:(jb + 1) * 128], transpose=True
            )
            psum_z = zpool.tile([128, SB], FP32, tag="z")
            nc.tensor.matmul(
                psum_z[:], sdst_arr[:, jb * 128:(jb + 1) * 128],
                ssrc_arr[:, r0:r0 + SB], start=True, stop=False,
            )
            t = spool.tile([128, SB], FP16, tag="t")
            if relu_act_frac > 0 and (jb % 16) / 16.0 < relu_act_frac:
                nc.scalar.activation(t[:], psum_z[:], AF.Relu, scale=4.0)
            else:
                nc.vector.tensor_scalar(
                    out=t[:], in0=psum_z[:], scalar1=4.0, scalar2=0.0,
                    op0=ALU.mult, op1=ALU.max,
                )
            nc.tensor.matmul(psum_z[:], i1_f16[:], t[:], start=False, stop=False)
            nc.tensor.matmul(psum_z[:], i500_bf[:], maskT[:], start=False, stop=True)
            em = mpool.tile([128, SB], BF16, tag="em")
            nc.scalar.activation(em[:], psum_z[:], AF.Exp, bias=neg500[:])
            nc.tensor.matmul(
                psum_hT[:F, :], wh_sb[:, jb * F:(jb + 1) * F], em[:],
                start=(jb == 0), stop=(jb == n_jb - 1),
            )
            nc.tensor.matmul(
                psum_den[:], ones_sb[:], em[:],
                start=(jb == 0), stop=(jb == n_jb - 1),
            )

        den_r = fpool.tile([1, SB], FP32, tag="denr")
        nc.vector.reciprocal(den_r[:], psum_den[:])
        hT_sb = fpool.tile([128, SB], BF16, tag="hTs")
        nc.scalar.copy(hT_sb[:F, :], psum_hT[:F, :])
        for k in range(SB // 128):
            psum_dc = eppool.tile([128, 1], FP32, tag="ep")
            nc.tensor.transpose(psum_dc[:], den_r[0:1, k * 128:(k + 1) * 128], one1_f32[:])
            rc = spool.tile([128, 1], FP32, tag="rc")
            nc.vector.tensor_copy(rc[:], psum_dc[:])
            psum_h = eppool.tile([128, F], BF16, tag="ep")
            nc.tensor.transpose(psum_h[:, :F], hT_sb[:F, k * 128:(k + 1) * 128], id_bf[:F, :F])
            out_cb(r0 + k * 128, psum_h, rc)


def _build(phase=99):
    import os
    phase = int(os.environ.get("K_PHASE", phase))
    nc = bacc.Bacc("TRN2", target_bir_lowering=False, debug=False, num_devices=NCORES)
    _build_body(nc, phase)
    nc.compile()
    return nc


def _build_body(nc, phase):

    # ---- external inputs (per-core) ----
    adjbf = nc.dram_tensor("adjbf", [RL, N], BF16, kind="ExternalInput").ap()
    xT = nc.dram_tensor("xT", [D, RL], FP32, kind="ExternalInput").ap()
    wpT = nc.dram_tensor("wpT", [D, H], FP32, kind="ExternalInput").ap()
    bp_d = nc.dram_tensor("bp", [H, 1], FP32, kind="ExternalInput").ap()
    w1T = nc.dram_tensor("w1T", [H, H], FP32, kind="ExternalInput").ap()
    w2T = nc.dram_tensor("w2T", [H, O], FP32, kind="ExternalInput").ap()
    a1c = nc.dram_tensor("a1c", [H, 2], FP32, kind="ExternalInput").ap()  # cols: [a1_src, a1_dst]
    a2c = nc.dram_tensor("a2c", [O, 2], FP32, kind="ExternalInput").ap()
    i500_d = nc.dram_tensor("i500", [128, 128], BF16, kind="ExternalInput").ap()
    i1_d = nc.dram_tensor("i1", [128, 128], FP16, kind="ExternalInput").ap()
    id_d = nc.dram_tensor("idm", [128, 128], BF16, kind="ExternalInput").ap()
    ones_row_d = nc.dram_tensor("ones_row", [1, N], FP16, kind="ExternalInput").ap()
    out_d = nc.dram_tensor("hout", [RL, O], FP32, kind="ExternalOutput").ap()

    # ---- collective buffers ----
    ag1_in = nc.dram_tensor("ag1_in", [RL, H], BF16, kind="Internal").ap()
    ag1_out = nc.dram_tensor("ag1_out", [N, H], BF16, kind="Internal", addr_space="Shared").ap()
    agS1_in = nc.dram_tensor("agS1_in", [1, RL], FP32, kind="Internal").ap()
    agS1_out = nc.dram_tensor("agS1_out", [NCORES, RL], FP32, kind="Internal", addr_space="Shared").ap()
    ag2_in = nc.dram_tensor("ag2_in", [RL, O], BF16, kind="Internal").ap()
    ag2_out = nc.dram_tensor("ag2_out", [N, O], BF16, kind="Internal", addr_space="Shared").ap()
    agS2_in = nc.dram_tensor("agS2_in", [1, RL], FP32, kind="Internal").ap()
    agS2_out = nc.dram_tensor("agS2_out", [NCORES, RL], FP32, kind="Internal", addr_space="Shared").ap()

    with TileContext(nc) as tc:
        with ExitStack() as ctx:
            cpool = ctx.enter_context(tc.tile_pool(name="const", bufs=1))
            prpool = ctx.enter_context(tc.tile_pool(name="pre_sb", bufs=2))
            epool = ctx.enter_context(tc.tile_pool(name="elup", bufs=2))
            outp = ctx.enter_context(tc.tile_pool(name="outp", bufs=2))
            ctx_pre = ctx.enter_context(ExitStack())
            ppool = ctx_pre.enter_context(tc.tile_pool(name="pre_ps", bufs=2, space="PSUM"))

            # constants
            i500_bf = cpool.tile([128, 128], BF16)
            nc.sync.dma_start(i500_bf[:], i500_d[:])
            i1_f16 = cpool.tile([128, 128], FP16)
            nc.sync.dma_start(i1_f16[:], i1_d[:])
            id_bf = cpool.tile([128, 128], BF16)
            nc.sync.dma_start(id_bf[:], id_d[:])
            ones_sb = cpool.tile([128, 1], BF16)
            nc.vector.memset(ones_sb[:], 1.0)
            one1_f32 = cpool.tile([1, 1], FP32)
            nc.vector.memset(one1_f32[:], 1.0)
            neg500 = cpool.tile([128, 1], FP32)
            nc.vector.memset(neg500[:], -MASK_BIG)
            bp_sb = cpool.tile([128, 1], FP32)
            nc.sync.dma_start(bp_sb[:], bp_d[:])
            wpT_sb = cpool.tile([128, 2 * H], FP32)  # d-chunks: [:, dk*H:(dk+1)*H]
            nc.sync.dma_start(
                wpT_sb[:].rearrange("p (b h) -> p b h", b=2),
                wpT.rearrange("(b p) h -> p b h", p=128),
            )
            w1T_sb = cpool.tile([128, H], FP32)
            nc.sync.dma_start(w1T_sb[:], w1T[:])
            w2T_sb = cpool.tile([128, O], FP32)
            nc.sync.dma_start(w2T_sb[:], w2T[:])
            a1c_sb = cpool.tile([128, 2], FP32)
            nc.sync.dma_start(a1c_sb[:], a1c[:])
            a2c_sb = cpool.tile([64, 2], FP32)
            nc.sync.dma_start(a2c_sb[:], a2c[:])

            # ---- preamble: hT_local = relu(WpT.T @ xT + bp)  [H=128, RL] ----
            xT_sb = cpool.tile([128, 2 * RL], FP32)  # d-chunks: [:, dk*RL:(dk+1)*RL]
            nc.sync.dma_start(
                xT_sb[:].rearrange("p (b r) -> p b r", b=2),
                xT.rearrange("(b p) r -> p b r", p=128),
            )
            hT_sb = cpool.tile([128, RL], FP32)
            for nk in range(RL // 512):
                ps = ppool.tile([128, 512], FP32, tag="pre")
                for dk in range(2):
                    nc.tensor.matmul(
                        ps[:], wpT_sb[:, dk * H:(dk + 1) * H],
                        xT_sb[:, dk * RL + nk * 512: dk * RL + (nk + 1) * 512],
                        start=(dk == 0), stop=(dk == 1),
                    )
                nc.scalar.activation(hT_sb[:, nk * 512:(nk + 1) * 512], ps[:], AF.Relu, bias=bp_sb[:])

            # ---- Wh1_local [n,F] (bf16, for AG) + Wh1T_local [F,n] (fp32, scores) ----
            wh1loc_bf = prpool.tile([128, 8 * H], BF16, tag="wh1l")
            for k in range(8):
                ps = ppool.tile([128, 512], FP32, tag="pre")
                nc.tensor.matmul(ps[:, :H], hT_sb[:, k * 128:(k + 1) * 128], w1T_sb[:], start=True, stop=True)
                nc.scalar.copy(wh1loc_bf[:, k * H:(k + 1) * H], ps[:, :H])
            nc.sync.dma_start(
                ag1_in.rearrange("(b p) f -> p b f", p=128),
                wh1loc_bf[:].rearrange("p (b f) -> p b f", b=8),
            )
            wh1T_sb = prpool.tile([128, RL], FP32, tag="wh1T")
            for nk in range(RL // 512):
                ps = ppool.tile([128, 512], FP32, tag="pre")
                nc.tensor.matmul(ps[:], w1T_sb[:], hT_sb[:, nk * 512:(nk + 1) * 512], start=True, stop=True)
                nc.scalar.copy(wh1T_sb[:, nk * 512:(nk + 1) * 512], ps[:])

            # ---- local scores: 0.2*s1_src (fp16 row), 0.2*s1_dst -> AG ----
            ssrc1_arr = cpool.tile([2, RL], FP16)   # row0 = 0.2*s_src, row1 = 1
            nc.sync.dma_start(ssrc1_arr[1:2, :], ones_row_d[0:1, :RL])
            sdst1_arr = cpool.tile([2, N], FP16)    # row0 = 1, row1 = 0.2*s_dst (j-order)
            nc.sync.dma_start(sdst1_arr[0:1, :], ones_row_d[0:1, :])
            for nk in range(RL // 512):
                pss = ppool.tile([1, 512], FP32, tag="ss")
                nc.tensor.matmul(pss[:], a1c_sb[:, 0:1], wh1T_sb[:, nk * 512:(nk + 1) * 512], start=True, stop=True)
                nc.vector.tensor_scalar(
                    out=ssrc1_arr[0:1, nk * 512:(nk + 1) * 512], in0=pss[:],
                    scalar1=0.2, scalar2=None, op0=ALU.mult,
                )
                psd = ppool.tile([1, 512], FP32, tag="ss")
                nc.tensor.matmul(psd[:], a1c_sb[:, 1:2], wh1T_sb[:, nk * 512:(nk + 1) * 512], start=True, stop=True)
                sd32 = prpool.tile([1, 512], FP32, tag="sd32")
                nc.vector.tensor_scalar(
                    out=sd32[:], in0=psd[:], scalar1=0.2, scalar2=None, op0=ALU.mult,
                )
                nc.sync.dma_start(agS1_in[0:1, nk * 512:(nk + 1) * 512], sd32[:])

            if phase <= 1:
                nc.sync.dma_start(out_d[0:128, :], hT_sb[:128, :O])
                return
            # ---- AllGathers for layer 1 ----
            nc.gpsimd.collective_compute(
                "AllGather", ALU.bypass, ins=[ag1_in[:]], outs=[ag1_out[:]],
                replica_groups=[list(range(NCORES))],
            )
            nc.gpsimd.collective_compute(
                "AllGather", ALU.bypass, ins=[agS1_in[:]], outs=[agS1_out[:]],
                replica_groups=[list(range(NCORES))],
            )
            wh_sb = cpool.tile([128, (N // 128) * H], BF16)
            nc.sync.dma_start(
                wh_sb[:].rearrange("p (b f) -> p b f", b=N // 128),
                ag1_out.rearrange("(b p) f -> p b f", p=128),
            )
            nc.gpsimd.dma_start(sdst1_arr[1:2, :], agS1_out.rearrange("a b -> (a b)")[None, :])

            if phase <= 2:
                nc.sync.dma_start(out_d[0:128, :], wh_sb[:128, :O])
                return
            # ---- layer-1 attention -> h1 (bf16 [r,F] chunks in h1_sb) ----
            h1_sb = prpool.tile([128, 8 * H], BF16, tag="h1")

            def out_cb1(r0, psum_h, rc):
                k = r0 // 128
                x = epool.tile([128, H], FP32, tag="elux")
                nc.vector.tensor_scalar(
                    out=x[:], in0=psum_h[:, :H], scalar1=rc[:], scalar2=None, op0=ALU.mult
                )
                a = epool.tile([128, H], FP32, tag="elua")
                nc.scalar.activation(a[:], psum_h[:, :H], AF.Exp, scale=rc[:])
                b = epool.tile([128, H], FP32, tag="elub")
                nc.vector.tensor_scalar(
                    out=b[:], in0=a[:], scalar1=-1.0, scalar2=0.0, op0=ALU.add, op1=ALU.min
                )
                nc.vector.tensor_tensor(
                    out=h1_sb[:, k * H:(k + 1) * H], in0=x[:], in1=b[:], op=ALU.max
                )

            ctx_pre.close()
            ctx_l1 = ctx.enter_context(ExitStack())
            _gat_attention(
                nc, tc, ctx_l1, name="l1", adjbf=adjbf, wh_sb=wh_sb, ones_sb=ones_sb,
                i500_bf=i500_bf, i1_f16=i1_f16, id_bf=id_bf, one1_f32=one1_f32,
                neg500=neg500, sdst_arr=sdst1_arr, ssrc_arr=ssrc1_arr,
                out_cb=out_cb1, R_LOCAL=RL, n=N, F=H,
            )
            ctx_l1.close()

            if phase <= 3:
                nc.sync.dma_start(out_d[0:128, :], h1_sb[:128, :O])
                return
            # ---- h1T (bf16) via PE transposes ----
            h1T_sb = prpool.tile([128, RL], BF16, tag="h1T")
            ctx_mid = ctx.enter_context(ExitStack())
            tpool = ctx_mid.enter_context(tc.tile_pool(name="tp", bufs=2, space="PSUM"))
            ppool = ctx_mid.enter_context(tc.tile_pool(name="mid_ps", bufs=2, space="PSUM"))
            for k in range(8):
                pt = tpool.tile([128, 128], BF16, tag="tp")
                nc.tensor.transpose(pt[:], h1_sb[:, k * H:(k + 1) * H], id_bf[:])
                nc.scalar.copy(h1T_sb[:, k * 128:(k + 1) * 128], pt[:])

            # ---- Wh2_local [n,O] bf16 -> AG ; Wh2T_local [O,n] ; scores s2 ----
            w2T_bf = cpool.tile([128, O], BF16)
            nc.vector.tensor_copy(w2T_bf[:], w2T_sb[:])
            wh2loc_bf = prpool.tile([128, 8 * O], BF16, tag="wh2l")
            for k in range(8):
                ps = ppool.tile([128, 512], FP32, tag="pre")
                nc.tensor.matmul(ps[:, :O], h1T_sb[:, k * 128:(k + 1) * 128], w2T_bf[:], start=True, stop=True)
                nc.scalar.copy(wh2loc_bf[:, k * O:(k + 1) * O], ps[:, :O])
            nc.sync.dma_start(
                ag2_in.rearrange("(b p) f -> p b f", p=128),
                wh2loc_bf[:].rearrange("p (b f) -> p b f", b=8),
            )
            wh2T_sb = prpool.tile([64, RL], FP32, tag="wh2T")
            for nk in range(RL // 512):
                ps = ppool.tile([128, 512], FP32, tag="pre")
                nc.tensor.matmul(ps[:64, :512], w2T_bf[:], h1T_sb[:, nk * 512:(nk + 1) * 512], start=True, stop=True)
                nc.scalar.copy(wh2T_sb[:, nk * 512:(nk + 1) * 512], ps[:64, :512])

            ssrc2_arr = cpool.tile([2, RL], FP16)
            nc.sync.dma_start(ssrc2_arr[1:2, :], ones_row_d[0:1, :RL])
            sdst2_arr = cpool.tile([2, N], FP16)
            nc.sync.dma_start(sdst2_arr[0:1, :], ones_row_d[0:1, :])
            for nk in range(RL // 512):
                pss = ppool.tile([1, 512], FP32, tag="ss")
                nc.tensor.matmul(pss[:], a2c_sb[:, 0:1], wh2T_sb[:, nk * 512:(nk + 1) * 512], start=True, stop=True)
                nc.vector.tensor_scalar(
                    out=ssrc2_arr[0:1, nk * 512:(nk + 1) * 512], in0=pss[:],
                    scalar1=0.2, scalar2=None, op0=ALU.mult,
                )
                psd = ppool.tile([1, 512], FP32, tag="ss")
                nc.tensor.matmul(psd[:], a2c_sb[:, 1:2], wh2T_sb[:, nk * 512:(nk + 1) * 512], start=True, stop=True)
                sd32 = prpool.tile([1, 512], FP32, tag="sd32")
                nc.vector.tensor_scalar(
                    out=sd32[:], in0=psd[:], scalar1=0.2, scalar2=None, op0=ALU.mult,
                )
                nc.sync.dma_start(agS2_in[0:1, nk * 512:(nk + 1) * 512], sd32[:])

            nc.gpsimd.collective_compute(
                "AllGather", ALU.bypass, ins=[ag2_in[:]], outs=[ag2_out[:]],
                replica_groups=[list(range(NCORES))],
            )
            nc.gpsimd.collective_compute(
                "AllGather", ALU.bypass, ins=[agS2_in[:]], outs=[agS2_out[:]],
                replica_groups=[list(range(NCORES))],
            )
            wh2_sb = cpool.tile([128, (N // 128) * O], BF16)
            nc.sync.dma_start(
                wh2_sb[:].rearrange("p (b f) -> p b f", b=N // 128),
                ag2_out.rearrange("(b p) f -> p b f", p=128),
            )
            nc.gpsimd.dma_start(sdst2_arr[1:2, :], agS2_out.rearrange("a b -> (a b)")[None, :])

            if phase <= 4:
                nc.sync.dma_start(out_d[0:128, :], wh2_sb[:128, :O])
                return
            # ---- layer-2 attention -> output (no ELU) ----
            def out_cb2(r0, psum_h, rc):
                o = outp.tile([128, O], FP32, tag="hout")
                nc.vector.tensor_scalar(
                    out=o[:], in0=psum_h[:, :O], scalar1=rc[:], scalar2=None, op0=ALU.mult
                )
                nc.sync.dma_start(out_d[r0:r0 + 128, :], o[:])

            ctx_mid.close()
            ctx_l2 = ctx.enter_context(ExitStack())
            _gat_attention(
                nc, tc, ctx_l2, name="l2", adjbf=adjbf, wh_sb=wh2_sb, ones_sb=ones_sb,
                i500_bf=i500_bf, i1_f16=i1_f16, id_bf=id_bf, one1_f32=one1_f32,
                neg500=neg500, sdst_arr=sdst2_arr, ssrc_arr=ssrc2_arr,
                out_cb=out_cb2, R_LOCAL=RL, n=N, F=O,
            )


def kernel(x, adj, Wp, bp, W1, a1, W2, a2):
    x = np.asarray(x); adj = np.asarray(adj)
    Wp = np.asarray(Wp, np.float32); bp = np.asarray(bp, np.float32)
    W1 = np.asarray(W1, np.float32); a1 = np.asarray(a1, np.float32)
    W2 = np.asarray(W2, np.float32); a2 = np.asarray(a2, np.float32)

    if "nc" not in _CACHED:
        _CACHED["nc"] = _build()
    nc = _CACHED["nc"]

    adjbf = (adj > 0).astype(ml_dtypes.bfloat16)          # [N, N]
    xT = np.ascontiguousarray(x.astype(np.float32).T)     # [D, N]
    shared = {
        "wpT": np.ascontiguousarray(Wp.T),                # [D, H]
        "bp": bp.reshape(H, 1),
        "w1T": np.ascontiguousarray(W1.T),                # [H, H]
        "w2T": np.ascontiguousarray(W2.T),                # [H, O]
        "a1c": np.stack([a1[0, :H], a1[0, H:]], axis=1),  # [H, 2]
        "a2c": np.stack([a2[0, :O], a2[0, O:]], axis=1),  # [O, 2]
        "i500": (np.eye(128) * MASK_BIG).astype(ml_dtypes.bfloat16),
        "i1": np.eye(128).astype(np.float16),
        "idm": np.eye(128).astype(ml_dtypes.bfloat16),
        "ones_row": np.ones((1, N), np.float16),
    }
    in_maps = []
    for c in range(NCORES):
        m = dict(shared)
        m["adjbf"] = adjbf[c * RL:(c + 1) * RL, :]
        m["xT"] = np.ascontiguousarray(xT[:, c * RL:(c + 1) * RL])
        in_maps.append(m)

    res = bass_utils.run_bass_kernel_spmd(nc, in_maps, core_ids=list(range(NCORES)))
    out = np.concatenate([res.results[c]["hout"] for c in range(NCORES)], axis=0)
    return out.astype(np.float32)


# revision 14
# speedup vs baseline: 1.2522x; 1.2522x over previous
"""Trainium2 Bass kernel for the 2-layer GAT message-passing network.

kernel(**inputs) takes FULL inputs (as from setup_inputs()) and returns the
FULL [8192, 64] float32 output. Work is sharded row-wise across 8 NeuronCores
(1024 attention rows each); [N,F] projections are shared via on-device
AllGather collectives between layers.

Per-layer attention ([j, r] tiles, orientation-2):
  psum_z = 0.2*z via K=2 fp16 matmul (lhsT rows [0.2*s_dst; 1] x rhs [1; 0.2*s_src])
  t = relu(0.2z) via one DVE tensor_scalar on an SBUF 0.2*s_src broadcast tile
      (per-partition scalar = 0.2*s_dst column) -> 2x DVE mode
  PE fold: psum_z += (4I).T @ t -> leakyrelu_0.2(z)
  em0 = Exp(psum_z) on ACT -> bf16 ; em = em0 * maskT on DVE (bf16 2x)
  maskT via one [128, 1024] bf16 transpose-DMA per j-block
  mm-A: hT[F, r-half] += Wh[jb].T @ em ; mm-B: den += ones.T @ em
  epilogue: transpose back, normalize by 1/den, ELU (layer 1).
"""
import numpy as np
import ml_dtypes

import concourse.bass as bass
import concourse.bacc as bacc
import concourse.mybir as mybir
from concourse import bass_utils
from concourse.tile import TileContext
from contextlib import ExitStack

FP32 = mybir.dt.float32
BF16 = mybir.dt.bfloat16
FP16 = mybir.dt.float16
AF = mybir.ActivationFunctionType
ALU = mybir.AluOpType

N, D, H, O = 8192, 256, 128, 64
NCORES = 8
RL = N // NCORES
MASK_BIG = 500.0

_CACHED = {}


def _gat_attention(
    nc, tc, ctx, *, name, adjbf, wh_sb, ones_sb, i500_bf, i4_f16, id_bf,
    one1_f32, neg500, sdst_arr, sdst_cols, z0row, z0b, out_cb, R_LOCAL, n, F,
    mask_dve_frac=1.0, zbufs=4, mbufs=5,
):
    n_jb = n // 128
    n_h = R_LOCAL // 512

    zpool = ctx.enter_context(tc.tile_pool(name=f"{name}_z", bufs=zbufs, space="PSUM"))
    opool = ctx.enter_context(tc.tile_pool(name=f"{name}_o", bufs=1, space="PSUM"))
    spool = ctx.enter_context(tc.tile_pool(name=f"{name}_s", bufs=4))
    mpool = ctx.enter_context(tc.tile_pool(name=f"{name}_m", bufs=mbufs))
    fpool = ctx.enter_context(tc.tile_pool(name=f"{name}_f", bufs=2))

    psum_hT = [opool.tile([128, 512], FP32, tag=f"hT{h}", name=f"{name}_hT{h}") for h in range(n_h)]
    psum_den = [opool.tile([1, 512], FP32, tag=f"den{h}", name=f"{name}_den{h}") for h in range(n_h)]

    for jb in range(n_jb):
        maskT = mpool.tile([128, R_LOCAL], BF16, tag="mask")
        dma_eng = nc.sync if jb % 2 == 0 else nc.scalar
        dma_eng.dma_start(
            maskT[:], adjbf[0:R_LOCAL, jb * 128:(jb + 1) * 128], transpose=True
        )
        mask_dve = mask_dve_frac > 0 and (jb % 16) / 16.0 < mask_dve_frac
        for h in range(n_h):
            hs = slice(h * 512, (h + 1) * 512)
            psum_z = zpool.tile([128, 512], FP32, tag="z")
            nc.tensor.matmul(
                psum_z[:], sdst_arr[:, jb * 128:(jb + 1) * 128],
                z0row[0:2, hs], start=True, stop=False,
            )
            # t = relu(0.2z) from the SBUF broadcast tile (2x DVE mode)
            t = spool.tile([128, 512], FP16, tag="t")
            nc.vector.tensor_scalar(
                out=t[:], in0=z0b[:, hs], scalar1=sdst_cols[:, jb:jb + 1],
                scalar2=0.0, op0=ALU.add, op1=ALU.max,
            )
            em = mpool.tile([128, 512], BF16, tag="em")
            if mask_dve:
                nc.tensor.matmul(psum_z[:], i4_f16[:], t[:], start=False, stop=True)
                em0 = mpool.tile([128, 512], BF16, tag="em0")
                nc.scalar.activation(em0[:], psum_z[:], AF.Exp)
                nc.vector.tensor_tensor(out=em[:], in0=em0[:], in1=maskT[:, hs], op=ALU.mult)
            else:
                nc.tensor.matmul(psum_z[:], i4_f16[:], t[:], start=False, stop=False)
                nc.tensor.matmul(psum_z[:], i500_bf[:], maskT[:, hs], start=False, stop=True)
                nc.scalar.activation(em[:], psum_z[:], AF.Exp, bias=neg500[:])
            nc.tensor.matmul(
                psum_hT[h][:F, :], wh_sb[:, jb * F:(jb + 1) * F], em[:],
                start=(jb == 0), stop=(jb == n_jb - 1),
            )
            nc.tensor.matmul(
                psum_den[h][:], ones_sb[:], em[:],
                start=(jb == 0), stop=(jb == n_jb - 1),
            )

    for h in range(n_h):
        den_r = fpool.tile([1, 512], FP32, tag="denr")
        nc.vector.reciprocal(den_r[:], psum_den[h][:])
        hT_sb = fpool.tile([128, 512], BF16, tag="hTs")
        nc.scalar.copy(hT_sb[:F, :], psum_hT[h][:F, :])
        for k in range(4):
            psum_dc = zpool.tile([128, 1], FP32, tag="z")
            nc.tensor.transpose(psum_dc[:], den_r[0:1, k * 128:(k + 1) * 128], one1_f32[:])
            rc = spool.tile([128, 1], FP32, tag="rc")
            nc.vector.tensor_copy(rc[:], psum_dc[:])
            psum_h = zpool.tile([128, F], BF16, tag="z")
            nc.tensor.transpose(psum_h[:, :F], hT_sb[:F, k * 128:(k + 1) * 128], id_bf[:F, :F])
            out_cb(h * 512 + k * 128, psum_h, rc)


def _build(phase=99):
    import os
    phase = int(os.environ.get("K_PHASE", phase))
    nc = bacc.Bacc("TRN2", target_bir_lowering=False, debug=False, num_devices=NCORES)

    # ---- external inputs ----
    adjbf = nc.dram_tensor("adjbf", [RL, N], BF16, kind="ExternalInput").ap()
    xT = nc.dram_tensor("xT", [D, RL], FP16, kind="ExternalInput").ap()
    wpT = nc.dram_tensor("wpT", [D, H], FP16, kind="ExternalInput").ap()
    bp_d = nc.dram_tensor("bp", [H, 1], FP32, kind="ExternalInput").ap()
    w1T = nc.dram_tensor("w1T", [H, H], FP16, kind="ExternalInput").ap()
    w1Tc = nc.dram_tensor("w1Tc", [H, H], FP16, kind="ExternalInput").ap()  # = W1
    w2T_bf_d = nc.dram_tensor("w2Tbf", [H, O], BF16, kind="ExternalInput").ap()
    w2c_bf_d = nc.dram_tensor("w2cbf", [H, O], BF16, kind="ExternalInput").ap()  # = W2.T (lhsT for Wh2T)
    a1c = nc.dram_tensor("a1c", [H, 2], FP16, kind="ExternalInput").ap()
    a2c = nc.dram_tensor("a2c", [O, 2], BF16, kind="ExternalInput").ap()
    i500_d = nc.dram_tensor("i500", [128, 128], BF16, kind="ExternalInput").ap()
    i4_d = nc.dram_tensor("i4", [128, 128], FP16, kind="ExternalInput").ap()
    id_d = nc.dram_tensor("idm", [128, 128], BF16, kind="ExternalInput").ap()
    ones_row_d = nc.dram_tensor("ones_row", [1, N], FP16, kind="ExternalInput").ap()
    out_d = nc.dram_tensor("hout", [RL, O], FP32, kind="ExternalOutput").ap()

    # ---- collective buffers ----
    ag1_in = nc.dram_tensor("ag1_in", [RL, H], BF16, kind="Internal").ap()
    ag1_out = nc.dram_tensor("ag1_out", [N, H], BF16, kind="Internal", addr_space="Shared").ap()
    agS1_in = nc.dram_tensor("agS1_in", [1, RL], FP16, kind="Internal").ap()
    agS1_out = nc.dram_tensor("agS1_out", [NCORES, RL], FP16, kind="Internal", addr_space="Shared").ap()
    ag2_in = nc.dram_tensor("ag2_in", [RL, O], BF16, kind="Internal").ap()
    ag2_out = nc.dram_tensor("ag2_out", [N, O], BF16, kind="Internal", addr_space="Shared").ap()
    agS2_in = nc.dram_tensor("agS2_in", [1, RL], FP16, kind="Internal").ap()
    agS2_out = nc.dram_tensor("agS2_out", [NCORES, RL], FP16, kind="Internal", addr_space="Shared").ap()

    with TileContext(nc) as tc:
        with ExitStack() as ctx:
            _build_body(nc, tc, ctx, phase, dict(locals()))
    nc.compile()
    return nc


def _build_body(nc, tc, ctx, phase, T):
    adjbf = T["adjbf"]; xT = T["xT"]; wpT = T["wpT"]; bp_d = T["bp_d"]
    w1T = T["w1T"]; w1Tc = T["w1Tc"]; w2T_bf_d = T["w2T_bf_d"]; w2c_bf_d = T["w2c_bf_d"]
    a1c = T["a1c"]; a2c = T["a2c"]; i500_d = T["i500_d"]; i4_d = T["i4_d"]
    id_d = T["id_d"]; ones_row_d = T["ones_row_d"]; out_d = T["out_d"]
    ag1_in = T["ag1_in"]; ag1_out = T["ag1_out"]; agS1_in = T["agS1_in"]; agS1_out = T["agS1_out"]
    ag2_in = T["ag2_in"]; ag2_out = T["ag2_out"]; agS2_in = T["agS2_in"]; agS2_out = T["agS2_out"]

    cpool = ctx.enter_context(tc.tile_pool(name="const", bufs=1))
    prpool = ctx.enter_context(tc.tile_pool(name="pre_sb", bufs=2))
    epool = ctx.enter_context(tc.tile_pool(name="elup", bufs=2))
    outp = ctx.enter_context(tc.tile_pool(name="outp", bufs=2))
    ctx_pre = ctx.enter_context(ExitStack())
    ppool = ctx_pre.enter_context(tc.tile_pool(name="pre_ps", bufs=2, space="PSUM"))

    # constants
    i500_bf = cpool.tile([128, 128], BF16)
    nc.sync.dma_start(i500_bf[:], i500_d[:])
    i4_f16 = cpool.tile([128, 128], FP16)
    nc.sync.dma_start(i4_f16[:], i4_d[:])
    id_bf = cpool.tile([128, 128], BF16)
    nc.sync.dma_start(id_bf[:], id_d[:])
    ones_sb = cpool.tile([128, 1], BF16)
    nc.vector.memset(ones_sb[:], 1.0)
    one1_f32 = cpool.tile([1, 1], FP32)
    nc.vector.memset(one1_f32[:], 1.0)
    neg500 = cpool.tile([128, 1], FP32)
    nc.vector.memset(neg500[:], -MASK_BIG)
    bp_sb = cpool.tile([128, 1], FP32)
    nc.sync.dma_start(bp_sb[:], bp_d[:])
    wpT_sb = cpool.tile([128, 2 * H], FP16)
    nc.sync.dma_start(
        wpT_sb[:].rearrange("p (b h) -> p b h", b=2),
        wpT.rearrange("(b p) h -> p b h", p=128),
    )
    w1T_sb = cpool.tile([128, H], FP16)
    nc.sync.dma_start(w1T_sb[:], w1T[:])
    w1Tc_sb = cpool.tile([128, H], FP16)
    nc.sync.dma_start(w1Tc_sb[:], w1Tc[:])
    w2T_bf = cpool.tile([128, O], BF16)
    nc.sync.dma_start(w2T_bf[:], w2T_bf_d[:])
    w2c_bf = cpool.tile([128, O], BF16)
    nc.sync.dma_start(w2c_bf[:], w2c_bf_d[:])
    a1c_sb = cpool.tile([128, 2], FP16)
    nc.sync.dma_start(a1c_sb[:], a1c[:])
    a2c_sb = cpool.tile([64, 2], BF16)
    nc.sync.dma_start(a2c_sb[:], a2c[:])

    # ---- preamble: hT_local = relu(WpT.T @ xT + bp)  [128H, RL] fp16 ----
    xT_sb = cpool.tile([128, 2 * RL], FP16)
    nc.sync.dma_start(
        xT_sb[:].rearrange("p (b r) -> p b r", b=2),
        xT.rearrange("(b p) r -> p b r", p=128),
    )
    hT_sb = cpool.tile([128, RL], FP16)
    for nk in range(RL // 512):
        ps = ppool.tile([128, 512], FP32, tag="pre")
        for dk in range(2):
            nc.tensor.matmul(
                ps[:], wpT_sb[:, dk * H:(dk + 1) * H],
                xT_sb[:, dk * RL + nk * 512: dk * RL + (nk + 1) * 512],
                start=(dk == 0), stop=(dk == 1),
            )
        nc.scalar.activation(hT_sb[:, nk * 512:(nk + 1) * 512], ps[:], AF.Relu, bias=bp_sb[:])

    # ---- Wh1_local [n,H] bf16 (AG payload) + Wh1T_local [H,n] fp16 (scores) ----
    wh1loc_bf = prpool.tile([128, 8 * H], BF16, tag="wh1l")
    for k in range(8):
        ps = ppool.tile([128, 512], FP32, tag="pre")
        nc.tensor.matmul(ps[:, :H], hT_sb[:, k * 128:(k + 1) * 128], w1T_sb[:], start=True, stop=True)
        nc.scalar.copy(wh1loc_bf[:, k * H:(k + 1) * H], ps[:, :H])
    nc.sync.dma_start(
        ag1_in.rearrange("(b p) f -> p b f", p=128),
        wh1loc_bf[:].rearrange("p (b f) -> p b f", b=8),
    )
    wh1T_sb = prpool.tile([128, RL], FP16, tag="wh1T")
    for nk in range(RL // 512):
        ps = ppool.tile([128, 512], FP32, tag="pre")
        nc.tensor.matmul(ps[:], w1Tc_sb[:], hT_sb[:, nk * 512:(nk + 1) * 512], start=True, stop=True)
        nc.scalar.copy(wh1T_sb[:, nk * 512:(nk + 1) * 512], ps[:])

    # ---- local scores layer 1: z0 tile rows [1; 0.2*s_src; bcast] + s_dst -> AG ----
    z0row1 = cpool.tile([2, RL], FP16)
    nc.sync.dma_start(z0row1[0:1, :], ones_row_d[0:1, :RL])
    z0b1 = cpool.tile([128, RL], FP16)
    ssrc_row1 = prpool.tile([1, RL], FP16, tag="ssrcrow")
    for nk in range(RL // 512):
        pss = ppool.tile([1, 512], FP32, tag="ss")
        nc.tensor.matmul(pss[:], a1c_sb[:, 0:1], wh1T_sb[:, nk * 512:(nk + 1) * 512], start=True, stop=True)
        nc.vector.tensor_scalar(
            out=ssrc_row1[0:1, nk * 512:(nk + 1) * 512], in0=pss[:],
            scalar1=0.2, scalar2=None, op0=ALU.mult,
        )
        psd = ppool.tile([1, 512], FP32, tag="ss")
        nc.tensor.matmul(psd[:], a1c_sb[:, 1:2], wh1T_sb[:, nk * 512:(nk + 1) * 512], start=True, stop=True)
        sd16 = prpool.tile([1, 512], FP16, tag="sd16")
        nc.vector.tensor_scalar(out=sd16[:], in0=psd[:], scalar1=0.2, scalar2=None, op0=ALU.mult)
        nc.sync.dma_start(agS1_in[0:1, nk * 512:(nk + 1) * 512], sd16[:])
    nc.sync.dma_start(z0row1[1:2, :], ssrc_row1[:])
    for nk in range(RL // 512):
        psb = ppool.tile([128, 512], FP32, tag="pre")
        nc.tensor.matmul(psb[:], z0row1[0:1, 0:128], ssrc_row1[0:1, nk * 512:(nk + 1) * 512], start=True, stop=True)
        nc.scalar.copy(z0b1[:, nk * 512:(nk + 1) * 512], psb[:])

    if phase <= 1:
        nc.gpsimd.dma_start(out_d[0:128, :], hT_sb[:128, :O])
        return

    # ---- AllGathers for layer 1 (scores first: they gate the z-mms) ----
    nc.gpsimd.collective_compute(
        "AllGather", ALU.bypass, ins=[agS1_in[:]], outs=[agS1_out[:]],
        replica_groups=[list(range(NCORES))],
    )
    nc.gpsimd.collective_compute(
        "AllGather", ALU.bypass, ins=[ag1_in[:]], outs=[ag1_out[:]],
        replica_groups=[list(range(NCORES))],
    )
    sdst1_arr = cpool.tile([2, N], FP16)
    nc.sync.dma_start(sdst1_arr[0:1, :], agS1_out.rearrange("a b -> (a b)")[None, :])
    nc.sync.dma_start(sdst1_arr[1:2, :], ones_row_d[0:1, :])
    sdc1_16 = prpool.tile([128, N // 128], FP16, tag="sdc16")
    nc.sync.dma_start(
        sdc1_16[:],
        agS1_out.rearrange("a b -> (a b)").rearrange("(b p) -> b p", p=128),
        transpose=True,
    )
    sdst1_cols = cpool.tile([128, N // 128], FP32)
    nc.vector.tensor_copy(sdst1_cols[:], sdc1_16[:])
    wh_sb = cpool.tile([128, (N // 128) * H], BF16)
    nc.sync.dma_start(
        wh_sb[:].rearrange("p (b f) -> p b f", b=N // 128),
        ag1_out.rearrange("(b p) f -> p b f", p=128),
    )

    if phase <= 2:
        nc.gpsimd.dma_start(out_d[0:128, :], wh_sb[:128, :O])
        return

    # ---- layer-1 attention -> h1 ----
    h1_sb = prpool.tile([128, 8 * H], BF16, tag="h1")

    def out_cb1(r0, psum_h, rc):
        k = r0 // 128
        x = epool.tile([128, H], FP32, tag="elux")
        nc.vector.tensor_scalar(
            out=x[:], in0=psum_h[:, :H], scalar1=rc[:], scalar2=None, op0=ALU.mult
        )
        a = epool.tile([128, H], FP32, tag="elua")
        nc.scalar.activation(a[:], psum_h[:, :H], AF.Exp, scale=rc[:])
        b = epool.tile([128, H], FP32, tag="elub")
        nc.vector.tensor_scalar(
            out=b[:], in0=a[:], scalar1=-1.0, scalar2=0.0, op0=ALU.add, op1=ALU.min
        )
        nc.vector.tensor_tensor(
            out=h1_sb[:, k * H:(k + 1) * H], in0=x[:], in1=b[:], op=ALU.max
        )

    ctx_pre.close()
    ctx_l1 = ctx.enter_context(ExitStack())
    _gat_attention(
        nc, tc, ctx_l1, name="l1", adjbf=adjbf, wh_sb=wh_sb, ones_sb=ones_sb,
        i500_bf=i500_bf, i4_f16=i4_f16, id_bf=id_bf, one1_f32=one1_f32,
        neg500=neg500, sdst_arr=sdst1_arr, sdst_cols=sdst1_cols,
        z0row=z0row1, z0b=z0b1, out_cb=out_cb1, R_LOCAL=RL, n=N, F=H,
    )
    ctx_l1.close()

    if phase <= 3:
        nc.gpsimd.dma_start(out_d[0:128, :], h1_sb[:128, :O])
        return

    # ---- h1T via PE transposes ----
    h1T_sb = prpool.tile([128, RL], BF16, tag="h1T")
    ctx_mid = ctx.enter_context(ExitStack())
    tpool = ctx_mid.enter_context(tc.tile_pool(name="tp", bufs=2, space="PSUM"))
    ppool = ctx_mid.enter_context(tc.tile_pool(name="mid_ps", bufs=2, space="PSUM"))
    for k in range(8):
        pt = tpool.tile([128, 128], BF16, tag="tp")
        nc.tensor.transpose(pt[:], h1_sb[:, k * H:(k + 1) * H], id_bf[:])
        nc.scalar.copy(h1T_sb[:, k * 128:(k + 1) * 128], pt[:])

    # ---- Wh2_local -> AG ; Wh2T_local ; scores s2 ----
    wh2loc_bf = prpool.tile([128, 8 * O], BF16, tag="wh2l")
    for k in range(8):
        ps = ppool.tile([128, 512], FP32, tag="pre")
        nc.tensor.matmul(ps[:, :O], h1T_sb[:, k * 128:(k + 1) * 128], w2T_bf[:], start=True, stop=True)
        nc.scalar.copy(wh2loc_bf[:, k * O:(k + 1) * O], ps[:, :O])
    nc.sync.dma_start(
        ag2_in.rearrange("(b p) f -> p b f", p=128),
        wh2loc_bf[:].rearrange("p (b f) -> p b f", b=8),
    )
    wh2T_sb = prpool.tile([64, RL], BF16, tag="wh2T")
    for nk in range(RL // 512):
        ps = ppool.tile([128, 512], FP32, tag="pre")
        nc.tensor.matmul(ps[:64, :512], w2c_bf[:], h1T_sb[:, nk * 512:(nk + 1) * 512], start=True, stop=True)
        nc.scalar.copy(wh2T_sb[:, nk * 512:(nk + 1) * 512], ps[:64, :512])

    z0row2 = cpool.tile([2, RL], FP16)
    nc.sync.dma_start(z0row2[0:1, :], ones_row_d[0:1, :RL])
    z0b2 = cpool.tile([128, RL], FP16)
    ssrc_row2 = prpool.tile([1, RL], FP16, tag="ssrcrow")
    for nk in range(RL // 512):
        pss = ppool.tile([1, 512], FP32, tag="ss")
        nc.tensor.matmul(pss[:], a2c_sb[:, 0:1], wh2T_sb[:, nk * 512:(nk + 1) * 512], start=True, stop=True)
        nc.vector.tensor_scalar(
            out=ssrc_row2[0:1, nk * 512:(nk + 1) * 512], in0=pss[:],
            scalar1=0.2, scalar2=None, op0=ALU.mult,
        )
        psd = ppool.tile([1, 512], FP32, tag="ss")
        nc.tensor.matmul(psd[:], a2c_sb[:, 1:2], wh2T_sb[:, nk * 512:(nk + 1) * 512], start=True, stop=True)
        sd16 = prpool.tile([1, 512], FP16, tag="sd16")
        nc.vector.tensor_scalar(out=sd16[:], in0=psd[:], scalar1=0.2, scalar2=None, op0=ALU.mult)
        nc.sync.dma_start(agS2_in[0:1, nk * 512:(nk + 1) * 512], sd16[:])
    nc.sync.dma_start(z0row2[1:2, :], ssrc_row2[:])
    for nk in range(RL // 512):
        psb = ppool.tile([128, 512], FP32, tag="pre")
        nc.tensor.matmul(psb[:], z0row2[0:1, 0:128], ssrc_row2[0:1, nk * 512:(nk + 1) * 512], start=True, stop=True)
        nc.scalar.copy(z0b2[:, nk * 512:(nk + 1) * 512], psb[:])

    nc.gpsimd.collective_compute(
        "AllGather", ALU.bypass, ins=[agS2_in[:]], outs=[agS2_out[:]],
        replica_groups=[list(range(NCORES))],
    )
    nc.gpsimd.collective_compute(
        "AllGather", ALU.bypass, ins=[ag2_in[:]], outs=[ag2_out[:]],
        replica_groups=[list(range(NCORES))],
    )
    sdst2_arr = cpool.tile([2, N], FP16)
    nc.sync.dma_start(sdst2_arr[0:1, :], agS2_out.rearrange("a b -> (a b)")[None, :])
    nc.sync.dma_start(sdst2_arr[1:2, :], ones_row_d[0:1, :])
    sdc2_16 = prpool.tile([128, N // 128], FP16, tag="sdc16")
    nc.sync.dma_start(
        sdc2_16[:],
        agS2_out.rearrange("a b -> (a b)").rearrange("(b p) -> b p", p=128),
        transpose=True,
    )
    sdst2_cols = cpool.tile([128, N // 128], FP32)
    nc.vector.tensor_copy(sdst2_cols[:], sdc2_16[:])
    wh2_sb = cpool.tile([128, (N // 128) * O], BF16)
    nc.sync.dma_start(
        wh2_sb[:].rearrange("p (b f) -> p b f", b=N // 128),
        ag2_out.rearrange("(b p) f -> p b f", p=128),
    )
    ctx_mid.close()

    if phase <= 4:
        nc.gpsimd.dma_start(out_d[0:128, :], wh2_sb[:128, :O])
        return

    # ---- layer-2 attention -> output ----
    def out_cb2(r0, psum_h, rc):
        o = outp.tile([128, O], FP32, tag="hout")
        nc.vector.tensor_scalar(
            out=o[:], in0=psum_h[:, :O], scalar1=rc[:], scalar2=None, op0=ALU.mult
        )
        nc.sync.dma_start(out_d[r0:r0 + 128, :], o[:])

    ctx_l2 = ctx.enter_context(ExitStack())
    _gat_attention(
        nc, tc, ctx_l2, name="l2", adjbf=adjbf, wh_sb=wh2_sb, ones_sb=ones_sb,
        i500_bf=i500_bf, i4_f16=i4_f16, id_bf=id_bf, one1_f32=one1_f32,
        neg500=neg500, sdst_arr=sdst2_arr, sdst_cols=sdst2_cols,
        z0row=z0row2, z0b=z0b2, out_cb=out_cb2, R_LOCAL=RL, n=N, F=O,
    )
    ctx_l2.close()


def kernel(x, adj, Wp, bp, W1, a1, W2, a2):
    x = np.asarray(x); adj = np.asarray(adj)
    Wp = np.asarray(Wp, np.float32); bp = np.asarray(bp, np.float32)
    W1 = np.asarray(W1, np.float32); a1 = np.asarray(a1, np.float32)
    W2 = np.asarray(W2, np.float32); a2 = np.asarray(a2, np.float32)

    if "nc" not in _CACHED:
        _CACHED["nc"] = _build()
    nc = _CACHED["nc"]

    adjbf = (adj > 0).astype(ml_dtypes.bfloat16)
    xTf = np.ascontiguousarray(x.astype(np.float32).T).astype(np.float16)
    shared = {
        "wpT": np.ascontiguousarray(Wp.T).astype(np.float16),
        "bp": bp.reshape(H, 1).astype(np.float32),
        "w1T": np.ascontiguousarray(W1.T).astype(np.float16),
        "w1Tc": np.ascontiguousarray(W1.T).astype(np.float16),  # lhsT for Wh1T = W1.T
        "w2Tbf": np.ascontiguousarray(W2.T).astype(ml_dtypes.bfloat16),
        "w2cbf": np.ascontiguousarray(W2.T).astype(ml_dtypes.bfloat16),  # lhsT for Wh2T = W2.T
        "a1c": np.stack([a1[0, :H], a1[0, H:]], axis=1).astype(np.float16),
        "a2c": np.stack([a2[0, :O], a2[0, O:]], axis=1).astype(ml_dtypes.bfloat16),
        "i500": (np.eye(128) * MASK_BIG).astype(ml_dtypes.bfloat16),
        "i4": (np.eye(128) * 4.0).astype(np.float16),
        "idm": np.eye(128).astype(ml_dtypes.bfloat16),
        "ones_row": np.ones((1, N), np.float16),
    }
    in_maps = []
    for c in range(NCORES):
        m = dict(shared)
        m["adjbf"] = adjbf[c * RL:(c + 1) * RL, :]
        m["xT"] = np.ascontiguousarray(xTf[:, c * RL:(c + 1) * RL])
        in_maps.append(m)

    res = bass_utils.run_bass_kernel_spmd(nc, in_maps, core_ids=list(range(NCORES)))
    out = np.concatenate([res.results[c]["hout"] for c in range(NCORES)], axis=0)
    return out.astype(np.float32)


# revision 17
# speedup vs baseline: 1.3852x; 1.1062x over previous
"""Trainium2 Bass kernel for the 2-layer GAT message-passing network.

kernel(**inputs) takes FULL inputs (as from setup_inputs()) and returns the
FULL [8192, 64] float32 output. Work is sharded row-wise across 8 NeuronCores
(1024 attention rows each); [N,F] projections are shared via on-device
AllGather collectives between layers.

Per-layer attention ([j, r] tiles, orientation-2):
  psum_z = 0.2*z via K=2 fp16 matmul (lhsT rows [0.2*s_dst; 1] x rhs [1; 0.2*s_src])
  t = relu(0.2z) via one DVE tensor_scalar on an SBUF 0.2*s_src broadcast tile
      (per-partition scalar = 0.2*s_dst column) -> 2x DVE mode
  PE fold: psum_z += (4I).T @ t -> leakyrelu_0.2(z)
  em0 = Exp(psum_z) on ACT -> bf16 ; em = em0 * maskT on DVE (bf16 2x)
  maskT via one [128, 1024] bf16 transpose-DMA per j-block
  mm-A: hT[F, r-half] += Wh[jb].T @ em ; mm-B: den += ones.T @ em
  epilogue: transpose back, normalize by 1/den, ELU (layer 1).
"""
import numpy as np
import ml_dtypes

import concourse.bass as bass
import concourse.bacc as bacc
import concourse.mybir as mybir
from concourse import bass_utils
from concourse.tile import TileContext
from contextlib import ExitStack

FP32 = mybir.dt.float32
BF16 = mybir.dt.bfloat16
FP16 = mybir.dt.float16
AF = mybir.ActivationFunctionType
ALU = mybir.AluOpType

N, D, H, O = 8192, 256, 128, 64
NCORES = 8
RL = N // NCORES
MASK_BIG = 500.0

_CACHED = {}


def _gat_attention(
    nc, tc, ctx, *, name, adjbf, wh_sb, expsd_col, i500_bf, i4_f16, id_bf,
    one1_f32, neg500, sdst_cols, z0b, out_cb, R_LOCAL, n, F,
    fuse_den=False, mask_dve_frac=1.0, zbufs=4, mbufs=5,
):
    """Attention with exp(0.2 s_dst) factored into pre-scaled Wh (wh_sb) and the
    denominator ones-vector (expsd_col); exp(0.2 s_src) cancels in num/den.
    psum_w = (4I).T @ t only, where t = relu(0.2z) from an SBUF broadcast tile.
    If fuse_den: wh_sb blocks are [128, F+1] with the scaled-ones column last."""
    n_jb = n // 128
    n_h = R_LOCAL // 512
    FW = F + 1 if fuse_den else F

    zpool = ctx.enter_context(tc.tile_pool(name=f"{name}_z", bufs=zbufs, space="PSUM"))
    opool = ctx.enter_context(tc.tile_pool(name=f"{name}_o", bufs=1, space="PSUM"))
    spool = ctx.enter_context(tc.tile_pool(name=f"{name}_s", bufs=4))
    mpool = ctx.enter_context(tc.tile_pool(name=f"{name}_m", bufs=mbufs))
    fpool = ctx.enter_context(tc.tile_pool(name=f"{name}_f", bufs=2))

    psum_hT = [opool.tile([128, 512], FP32, tag=f"hT{h}", name=f"{name}_hT{h}") for h in range(n_h)]
    if not fuse_den:
        psum_den = [opool.tile([1, 512], FP32, tag=f"den{h}", name=f"{name}_den{h}") for h in range(n_h)]

    for jb in range(n_jb):
        maskT = mpool.tile([128, R_LOCAL], BF16, tag="mask")
        dma_eng = nc.sync if jb % 2 == 0 else nc.scalar
        dma_eng.dma_start(
            maskT[:], adjbf[0:R_LOCAL, jb * 128:(jb + 1) * 128], transpose=True
        )
        mask_dve = mask_dve_frac > 0 and (jb % 16) / 16.0 < mask_dve_frac
        for h in range(n_h):
            hs = slice(h * 512, (h + 1) * 512)
            t = spool.tile([128, 512], FP16, tag="t")
            nc.vector.tensor_scalar(
                out=t[:], in0=z0b[:, hs], scalar1=sdst_cols[:, jb:jb + 1],
                scalar2=0.0, op0=ALU.add, op1=ALU.max,
            )
            psum_z = zpool.tile([128, 512], FP32, tag="z")
            em = mpool.tile([128, 512], BF16, tag="em")
            if mask_dve:
                nc.tensor.matmul(psum_z[:], i4_f16[:], t[:], start=True, stop=True)
                em0 = mpool.tile([128, 512], BF16, tag="em0")
                nc.scalar.activation(em0[:], psum_z[:], AF.Exp)
                nc.vector.tensor_tensor(out=em[:], in0=em0[:], in1=maskT[:, hs], op=ALU.mult)
            else:
                nc.tensor.matmul(psum_z[:], i4_f16[:], t[:], start=True, stop=False)
                nc.tensor.matmul(psum_z[:], i500_bf[:], maskT[:, hs], start=False, stop=True)
                nc.scalar.activation(em[:], psum_z[:], AF.Exp, bias=neg500[:])
            nc.tensor.matmul(
                psum_hT[h][:FW, :], wh_sb[:, jb * FW:(jb + 1) * FW], em[:],
                start=(jb == 0), stop=(jb == n_jb - 1),
            )
            if not fuse_den:
                nc.tensor.matmul(
                    psum_den[h][:], expsd_col[:, jb:jb + 1], em[:],
                    start=(jb == 0), stop=(jb == n_jb - 1),
                )

    for h in range(n_h):
        den_r = fpool.tile([1, 512], FP32, tag="denr")
        if fuse_den:
            nc.vector.reciprocal(den_r[:], psum_hT[h][F:F + 1, :])
        else:
            nc.vector.reciprocal(den_r[:], psum_den[h][:])
        hT_sb = fpool.tile([128, 512], BF16, tag="hTs")
        nc.scalar.copy(hT_sb[:F, :], psum_hT[h][:F, :])
        for k in range(4):
            psum_dc = zpool.tile([128, 1], FP32, tag="z")
            nc.tensor.transpose(psum_dc[:], den_r[0:1, k * 128:(k + 1) * 128], one1_f32[:])
            rc = spool.tile([128, 1], FP32, tag="rc")
            nc.vector.tensor_copy(rc[:], psum_dc[:])
            psum_h = zpool.tile([128, F], BF16, tag="z")
            nc.tensor.transpose(psum_h[:, :F], hT_sb[:F, k * 128:(k + 1) * 128], id_bf[:F, :F])
            out_cb(h * 512 + k * 128, psum_h, rc)


def _build(phase=99):
    import os
    phase = int(os.environ.get("K_PHASE", phase))
    nc = bacc.Bacc("TRN2", target_bir_lowering=False, debug=False, num_devices=NCORES)

    # ---- external inputs ----
    adjbf = nc.dram_tensor("adjbf", [RL, N], BF16, kind="ExternalInput").ap()
    xT = nc.dram_tensor("xT", [D, RL], FP16, kind="ExternalInput").ap()
    wpT = nc.dram_tensor("wpT", [D, H], FP16, kind="ExternalInput").ap()
    bp_d = nc.dram_tensor("bp", [H, 1], FP32, kind="ExternalInput").ap()
    w1T = nc.dram_tensor("w1T", [H, H], FP16, kind="ExternalInput").ap()
    w1Tc = nc.dram_tensor("w1Tc", [H, H], FP16, kind="ExternalInput").ap()  # = W1
    w2T_bf_d = nc.dram_tensor("w2Tbf", [H, O], BF16, kind="ExternalInput").ap()
    w2c_bf_d = nc.dram_tensor("w2cbf", [H, O], BF16, kind="ExternalInput").ap()  # = W2.T (lhsT for Wh2T)
    a1c = nc.dram_tensor("a1c", [H, 2], FP16, kind="ExternalInput").ap()
    a2c = nc.dram_tensor("a2c", [O, 2], BF16, kind="ExternalInput").ap()
    i500_d = nc.dram_tensor("i500", [128, 128], BF16, kind="ExternalInput").ap()
    i4_d = nc.dram_tensor("i4", [128, 128], FP16, kind="ExternalInput").ap()
    id_d = nc.dram_tensor("idm", [128, 128], BF16, kind="ExternalInput").ap()
    ones_row_d = nc.dram_tensor("ones_row", [1, N], FP16, kind="ExternalInput").ap()
    out_d = nc.dram_tensor("hout", [RL, O], FP32, kind="ExternalOutput").ap()

    # ---- collective buffers ----
    ag1_in = nc.dram_tensor("ag1_in", [RL, H], BF16, kind="Internal").ap()
    ag1_out = nc.dram_tensor("ag1_out", [N, H], BF16, kind="Internal", addr_space="Shared").ap()
    agS1_in = nc.dram_tensor("agS1_in", [1, RL], FP16, kind="Internal").ap()
    agS1_out = nc.dram_tensor("agS1_out", [NCORES, RL], FP16, kind="Internal", addr_space="Shared").ap()
    ag2_in = nc.dram_tensor("ag2_in", [RL, O], BF16, kind="Internal").ap()
    ag2_out = nc.dram_tensor("ag2_out", [N, O], BF16, kind="Internal", addr_space="Shared").ap()
    agS2_in = nc.dram_tensor("agS2_in", [1, RL], FP16, kind="Internal").ap()
    agS2_out = nc.dram_tensor("agS2_out", [NCORES, RL], FP16, kind="Internal", addr_space="Shared").ap()

    with TileContext(nc) as tc:
        with ExitStack() as ctx:
            _build_body(nc, tc, ctx, phase, dict(locals()))
    nc.compile()
    return nc


def _build_body(nc, tc, ctx, phase, T):
    adjbf = T["adjbf"]; xT = T["xT"]; wpT = T["wpT"]; bp_d = T["bp_d"]
    w1T = T["w1T"]; w1Tc = T["w1Tc"]; w2T_bf_d = T["w2T_bf_d"]; w2c_bf_d = T["w2c_bf_d"]
    a1c = T["a1c"]; a2c = T["a2c"]; i500_d = T["i500_d"]; i4_d = T["i4_d"]
    id_d = T["id_d"]; ones_row_d = T["ones_row_d"]; out_d = T["out_d"]
    ag1_in = T["ag1_in"]; ag1_out = T["ag1_out"]; agS1_in = T["agS1_in"]; agS1_out = T["agS1_out"]
    ag2_in = T["ag2_in"]; ag2_out = T["ag2_out"]; agS2_in = T["agS2_in"]; agS2_out = T["agS2_out"]

    cpool = ctx.enter_context(tc.tile_pool(name="const", bufs=1))
    prpool = ctx.enter_context(tc.tile_pool(name="pre_sb", bufs=2))
    epool = ctx.enter_context(tc.tile_pool(name="elup", bufs=2))
    outp = ctx.enter_context(tc.tile_pool(name="outp", bufs=2))
    ctx_pre = ctx.enter_context(ExitStack())
    ppool = ctx_pre.enter_context(tc.tile_pool(name="pre_ps", bufs=2, space="PSUM"))

    # constants
    i500_bf = cpool.tile([128, 128], BF16)
    nc.sync.dma_start(i500_bf[:], i500_d[:])
    i4_f16 = cpool.tile([128, 128], FP16)
    nc.sync.dma_start(i4_f16[:], i4_d[:])
    id_bf = cpool.tile([128, 128], BF16)
    nc.sync.dma_start(id_bf[:], id_d[:])
    ones_sb = cpool.tile([128, 1], BF16)
    nc.vector.memset(ones_sb[:], 1.0)
    one1_f32 = cpool.tile([1, 1], FP32)
    nc.vector.memset(one1_f32[:], 1.0)
    neg500 = cpool.tile([128, 1], FP32)
    nc.vector.memset(neg500[:], -MASK_BIG)
    bp_sb = cpool.tile([128, 1], FP32)
    nc.sync.dma_start(bp_sb[:], bp_d[:])
    wpT_sb = cpool.tile([128, 2 * H], FP16)
    nc.sync.dma_start(
        wpT_sb[:].rearrange("p (b h) -> p b h", b=2),
        wpT.rearrange("(b p) h -> p b h", p=128),
    )
    w1T_sb = cpool.tile([128, H], FP16)
    nc.sync.dma_start(w1T_sb[:], w1T[:])
    w1Tc_sb = cpool.tile([128, H], FP16)
    nc.sync.dma_start(w1Tc_sb[:], w1Tc[:])
    w2T_bf = cpool.tile([128, O], BF16)
    nc.sync.dma_start(w2T_bf[:], w2T_bf_d[:])
    w2c_bf = cpool.tile([128, O], BF16)
    nc.sync.dma_start(w2c_bf[:], w2c_bf_d[:])
    a1c_sb = cpool.tile([128, 2], FP16)
    nc.sync.dma_start(a1c_sb[:], a1c[:])
    a2c_sb = cpool.tile([64, 2], BF16)
    nc.sync.dma_start(a2c_sb[:], a2c[:])

    # ---- preamble: hT_local = relu(WpT.T @ xT + bp)  [128H, RL] fp16 ----
    xT_sb = cpool.tile([128, 2 * RL], FP16)
    nc.sync.dma_start(
        xT_sb[:].rearrange("p (b r) -> p b r", b=2),
        xT.rearrange("(b p) r -> p b r", p=128),
    )
    hT_sb = cpool.tile([128, RL], FP16)
    for nk in range(RL // 512):
        ps = ppool.tile([128, 512], FP32, tag="pre")
        for dk in range(2):
            nc.tensor.matmul(
                ps[:], wpT_sb[:, dk * H:(dk + 1) * H],
                xT_sb[:, dk * RL + nk * 512: dk * RL + (nk + 1) * 512],
                start=(dk == 0), stop=(dk == 1),
            )
        nc.scalar.activation(hT_sb[:, nk * 512:(nk + 1) * 512], ps[:], AF.Relu, bias=bp_sb[:])

    # ---- Wh1_local [n,H] bf16 (AG payload) + Wh1T_local [H,n] fp16 (scores) ----
    wh1loc_bf = prpool.tile([128, 8 * H], BF16, tag="wh1l")
    for k in range(8):
        ps = ppool.tile([128, 512], FP32, tag="pre")
        nc.tensor.matmul(ps[:, :H], hT_sb[:, k * 128:(k + 1) * 128], w1T_sb[:], start=True, stop=True)
        nc.scalar.copy(wh1loc_bf[:, k * H:(k + 1) * H], ps[:, :H])
    nc.sync.dma_start(
        ag1_in.rearrange("(b p) f -> p b f", p=128),
        wh1loc_bf[:].rearrange("p (b f) -> p b f", b=8),
    )
    wh1T_sb = prpool.tile([128, RL], FP16, tag="wh1T")
    for nk in range(RL // 512):
        ps = ppool.tile([128, 512], FP32, tag="pre")
        nc.tensor.matmul(ps[:], w1Tc_sb[:], hT_sb[:, nk * 512:(nk + 1) * 512], start=True, stop=True)
        nc.scalar.copy(wh1T_sb[:, nk * 512:(nk + 1) * 512], ps[:])

    # ---- local scores layer 1: z0 tile rows [1; 0.2*s_src; bcast] + s_dst -> AG ----
    z0row1 = cpool.tile([2, RL], FP16)
    nc.sync.dma_start(z0row1[0:1, :], ones_row_d[0:1, :RL])
    z0b1 = cpool.tile([128, RL], FP16)
    ssrc_row1 = prpool.tile([1, RL], FP16, tag="ssrcrow")
    for nk in range(RL // 512):
        pss = ppool.tile([1, 512], FP32, tag="ss")
        nc.tensor.matmul(pss[:], a1c_sb[:, 0:1], wh1T_sb[:, nk * 512:(nk + 1) * 512], start=True, stop=True)
        nc.vector.tensor_scalar(
            out=ssrc_row1[0:1, nk * 512:(nk + 1) * 512], in0=pss[:],
            scalar1=0.2, scalar2=None, op0=ALU.mult,
        )
        psd = ppool.tile([1, 512], FP32, tag="ss")
        nc.tensor.matmul(psd[:], a1c_sb[:, 1:2], wh1T_sb[:, nk * 512:(nk + 1) * 512], start=True, stop=True)
        sd16 = prpool.tile([1, 512], FP16, tag="sd16")
        nc.vector.tensor_scalar(out=sd16[:], in0=psd[:], scalar1=0.2, scalar2=None, op0=ALU.mult)
        nc.sync.dma_start(agS1_in[0:1, nk * 512:(nk + 1) * 512], sd16[:])
    nc.sync.dma_start(z0row1[1:2, :], ssrc_row1[:])
    for nk in range(RL // 512):
        psb = ppool.tile([128, 512], FP32, tag="pre")
        nc.tensor.matmul(psb[:], z0row1[0:1, 0:128], ssrc_row1[0:1, nk * 512:(nk + 1) * 512], start=True, stop=True)
        nc.scalar.copy(z0b1[:, nk * 512:(nk + 1) * 512], psb[:])

    if phase <= 1:
        nc.gpsimd.dma_start(out_d[0:128, :], hT_sb[:128, :O])
        return

    # ---- AllGathers for layer 1 (scores first: they gate the z-mms) ----
    nc.gpsimd.collective_compute(
        "AllGather", ALU.bypass, ins=[agS1_in[:]], outs=[agS1_out[:]],
        replica_groups=[list(range(NCORES))],
    )
    nc.gpsimd.collective_compute(
        "AllGather", ALU.bypass, ins=[ag1_in[:]], outs=[ag1_out[:]],
        replica_groups=[list(range(NCORES))],
    )
    sdc1_16 = prpool.tile([128, N // 128], FP16, tag="sdc16")
    nc.sync.dma_start(
        sdc1_16[:],
        agS1_out.rearrange("a b -> (a b)").rearrange("(b p) -> b p", p=128),
        transpose=True,
    )
    sdst1_cols = cpool.tile([128, N // 128], FP32)
    nc.vector.tensor_copy(sdst1_cols[:], sdc1_16[:])
    expsd1_f32 = cpool.tile([128, N // 128], FP32)
    nc.scalar.activation(expsd1_f32[:], sdst1_cols[:], AF.Exp)
    expsd1_bf = cpool.tile([128, N // 128], BF16)
    nc.vector.tensor_copy(expsd1_bf[:], expsd1_f32[:])
    wh_raw = prpool.tile([128, (N // 128) * H], BF16, tag="whraw")
    nc.sync.dma_start(
        wh_raw[:].rearrange("p (b f) -> p b f", b=N // 128),
        ag1_out.rearrange("(b p) f -> p b f", p=128),
    )
    wh_sb = cpool.tile([128, (N // 128) * H], BF16)
    for b in range(N // 128):
        nc.vector.tensor_scalar(
            out=wh_sb[:, b * H:(b + 1) * H], in0=wh_raw[:, b * H:(b + 1) * H],
            scalar1=expsd1_f32[:, b:b + 1], scalar2=None, op0=ALU.mult,
        )

    if phase <= 2:
        nc.gpsimd.dma_start(out_d[0:128, :], wh_sb[:128, :O])
        return

    # ---- layer-1 attention -> h1 ----
    h1_sb = prpool.tile([128, 8 * H], BF16, tag="h1")

    def out_cb1(r0, psum_h, rc):
        k = r0 // 128
        x = epool.tile([128, H], FP32, tag="elux")
        nc.vector.tensor_scalar(
            out=x[:], in0=psum_h[:, :H], scalar1=rc[:], scalar2=None, op0=ALU.mult
        )
        a = epool.tile([128, H], FP32, tag="elua")
        nc.scalar.activation(a[:], psum_h[:, :H], AF.Exp, scale=rc[:])
        b = epool.tile([128, H], FP32, tag="elub")
        nc.vector.tensor_scalar(
            out=b[:], in0=a[:], scalar1=-1.0, scalar2=0.0, op0=ALU.add, op1=ALU.min
        )
        nc.vector.tensor_tensor(
            out=h1_sb[:, k * H:(k + 1) * H], in0=x[:], in1=b[:], op=ALU.max
        )

    ctx_pre.close()
    ctx_l1 = ctx.enter_context(ExitStack())
    _gat_attention(
        nc, tc, ctx_l1, name="l1", adjbf=adjbf, wh_sb=wh_sb, expsd_col=expsd1_bf,
        i500_bf=i500_bf, i4_f16=i4_f16, id_bf=id_bf, one1_f32=one1_f32,
        neg500=neg500, sdst_cols=sdst1_cols,
        z0b=z0b1, out_cb=out_cb1, R_LOCAL=RL, n=N, F=H,
    )
    ctx_l1.close()

    if phase <= 3:
        nc.gpsimd.dma_start(out_d[0:128, :], h1_sb[:128, :O])
        return

    # ---- h1T via PE transposes ----
    h1T_sb = prpool.tile([128, RL], BF16, tag="h1T")
    ctx_mid = ctx.enter_context(ExitStack())
    tpool = ctx_mid.enter_context(tc.tile_pool(name="tp", bufs=2, space="PSUM"))
    ppool = ctx_mid.enter_context(tc.tile_pool(name="mid_ps", bufs=2, space="PSUM"))
    for k in range(8):
        pt = tpool.tile([128, 128], BF16, tag="tp")
        nc.tensor.transpose(pt[:], h1_sb[:, k * H:(k + 1) * H], id_bf[:])
        nc.scalar.copy(h1T_sb[:, k * 128:(k + 1) * 128], pt[:])

    # ---- Wh2_local -> AG ; Wh2T_local ; scores s2 ----
    wh2loc_bf = prpool.tile([128, 8 * O], BF16, tag="wh2l")
    for k in range(8):
        ps = ppool.tile([128, 512], FP32, tag="pre")
        nc.tensor.matmul(ps[:, :O], h1T_sb[:, k * 128:(k + 1) * 128], w2T_bf[:], start=True, stop=True)
        nc.scalar.copy(wh2loc_bf[:, k * O:(k + 1) * O], ps[:, :O])
    nc.sync.dma_start(
        ag2_in.rearrange("(b p) f -> p b f", p=128),
        wh2loc_bf[:].rearrange("p (b f) -> p b f", b=8),
    )
    wh2T_sb = prpool.tile([64, RL], BF16, tag="wh2T")
    for nk in range(RL // 512):
        ps = ppool.tile([128, 512], FP32, tag="pre")
        nc.tensor.matmul(ps[:64, :512], w2c_bf[:], h1T_sb[:, nk * 512:(nk + 1) * 512], start=True, stop=True)
        nc.scalar.copy(wh2T_sb[:, nk * 512:(nk + 1) * 512], ps[:64, :512])

    z0row2 = cpool.tile([2, RL], FP16)
    nc.sync.dma_start(z0row2[0:1, :], ones_row_d[0:1, :RL])
    z0b2 = cpool.tile([128, RL], FP16)
    ssrc_row2 = prpool.tile([1, RL], FP16, tag="ssrcrow")
    for nk in range(RL // 512):
        pss = ppool.tile([1, 512], FP32, tag="ss")
        nc.tensor.matmul(pss[:], a2c_sb[:, 0:1], wh2T_sb[:, nk * 512:(nk + 1) * 512], start=True, stop=True)
        nc.vector.tensor_scalar(
            out=ssrc_row2[0:1, nk * 512:(nk + 1) * 512], in0=pss[:],
            scalar1=0.2, scalar2=None, op0=ALU.mult,
        )
        psd = ppool.tile([1, 512], FP32, tag="ss")
        nc.tensor.matmul(psd[:], a2c_sb[:, 1:2], wh2T_sb[:, nk * 512:(nk + 1) * 512], start=True, stop=True)
        sd16 = prpool.tile([1, 512], FP16, tag="sd16")
        nc.vector.tensor_scalar(out=sd16[:], in0=psd[:], scalar1=0.2, scalar2=None, op0=ALU.mult)
        nc.sync.dma_start(agS2_in[0:1, nk * 512:(nk + 1) * 512], sd16[:])
    nc.sync.dma_start(z0row2[1:2, :], ssrc_row2[:])
    for nk in range(RL // 512):
        psb = ppool.tile([128, 512], FP32, tag="pre")
        nc.tensor.matmul(psb[:], z0row2[0:1, 0:128], ssrc_row2[0:1, nk * 512:(nk + 1) * 512], start=True, stop=True)
        nc.scalar.copy(z0b2[:, nk * 512:(nk + 1) * 512], psb[:])

    nc.gpsimd.collective_compute(
        "AllGather", ALU.bypass, ins=[agS2_in[:]], outs=[agS2_out[:]],
        replica_groups=[list(range(NCORES))],
    )
    nc.gpsimd.collective_compute(
        "AllGather", ALU.bypass, ins=[ag2_in[:]], outs=[ag2_out[:]],
        replica_groups=[list(range(NCORES))],
    )
    sdc2_16 = prpool.tile([128, N // 128], FP16, tag="sdc16")
    nc.sync.dma_start(
        sdc2_16[:],
        agS2_out.rearrange("a b -> (a b)").rearrange("(b p) -> b p", p=128),
        transpose=True,
    )
    sdst2_cols = cpool.tile([128, N // 128], FP32)
    nc.vector.tensor_copy(sdst2_cols[:], sdc2_16[:])
    expsd2_f32 = cpool.tile([128, N // 128], FP32)
    nc.scalar.activation(expsd2_f32[:], sdst2_cols[:], AF.Exp)
    expsd2_bf = cpool.tile([128, N // 128], BF16)
    nc.vector.tensor_copy(expsd2_bf[:], expsd2_f32[:])
    wh2_raw = prpool.tile([128, (N // 128) * (O + 1)], BF16, tag="wh2raw")
    nc.vector.memset(wh2_raw[:].rearrange("p (b f) -> p b f", f=O + 1)[:, :, O:O + 1], 1.0)
    nc.sync.dma_start(
        wh2_raw[:].rearrange("p (b f) -> p b f", f=O + 1)[:, :, 0:O],
        ag2_out.rearrange("(b p) f -> p b f", p=128),
    )
    wh2_sb = cpool.tile([128, (N // 128) * (O + 1)], BF16)
    for b in range(N // 128):
        nc.vector.tensor_scalar(
            out=wh2_sb[:, b * (O + 1):(b + 1) * (O + 1)],
            in0=wh2_raw[:, b * (O + 1):(b + 1) * (O + 1)],
            scalar1=expsd2_f32[:, b:b + 1], scalar2=None, op0=ALU.mult,
        )
    ctx_mid.close()

    if phase <= 4:
        nc.gpsimd.dma_start(out_d[0:128, :], wh2_sb[:128, :O])
        return

    # ---- layer-2 attention -> output ----
    def out_cb2(r0, psum_h, rc):
        o = outp.tile([128, O], FP32, tag="hout")
        nc.vector.tensor_scalar(
            out=o[:], in0=psum_h[:, :O], scalar1=rc[:], scalar2=None, op0=ALU.mult
        )
        nc.sync.dma_start(out_d[r0:r0 + 128, :], o[:])

    ctx_l2 = ctx.enter_context(ExitStack())
    _gat_attention(
        nc, tc, ctx_l2, name="l2", adjbf=adjbf, wh_sb=wh2_sb, expsd_col=expsd2_bf,
        i500_bf=i500_bf, i4_f16=i4_f16, id_bf=id_bf, one1_f32=one1_f32,
        neg500=neg500, sdst_cols=sdst2_cols,
        z0b=z0b2, out_cb=out_cb2, R_LOCAL=RL, n=N, F=O, fuse_den=True,
    )
    ctx_l2.close()


def kernel(x, adj, Wp, bp, W1, a1, W2, a2):
    x = np.asarray(x); adj = np.asarray(adj)
    Wp = np.asarray(Wp, np.float32); bp = np.asarray(bp, np.float32)
    W1 = np.asarray(W1, np.float32); a1 = np.asarray(a1, np.float32)
    W2 = np.asarray(W2, np.float32); a2 = np.asarray(a2, np.float32)

    if "nc" not in _CACHED:
        _CACHED["nc"] = _build()
    nc = _CACHED["nc"]

    adjbf = (adj > 0).astype(ml_dtypes.bfloat16)
    xTf = np.ascontiguousarray(x.astype(np.float32).T).astype(np.float16)
    shared = {
        "wpT": np.ascontiguousarray(Wp.T).astype(np.float16),
        "bp": bp.reshape(H, 1).astype(np.float32),
        "w1T": np.ascontiguousarray(W1.T).astype(np.float16),
        "w1Tc": np.ascontiguousarray(W1.T).astype(np.float16),  # lhsT for Wh1T = W1.T
        "w2Tbf": np.ascontiguousarray(W2.T).astype(ml_dtypes.bfloat16),
        "w2cbf": np.ascontiguousarray(W2.T).astype(ml_dtypes.bfloat16),  # lhsT for Wh2T = W2.T
        "a1c": np.stack([a1[0, :H], a1[0, H:]], axis=1).astype(np.float16),
        "a2c": np.stack([a2[0, :O], a2[0, O:]], axis=1).astype(ml_dtypes.bfloat16),
        "i500": (np.eye(128) * MASK_BIG).astype(ml_dtypes.bfloat16),
        "i4": (np.eye(128) * 4.0).astype(np.float16),
        "idm": np.eye(128).astype(ml_dtypes.bfloat16),
        "ones_row": np.ones((1, N), np.float16),
    }
    in_maps = []
    for c in range(NCORES):
        m = dict(shared)
        m["adjbf"] = adjbf[c * RL:(c + 1) * RL, :]
        m["xT"] = np.ascontiguousarray(xTf[:, c * RL:(c + 1) * RL])
        in_maps.append(m)

    res = bass_utils.run_bass_kernel_spmd(nc, in_maps, core_ids=list(range(NCORES)))
    out = np.concatenate([res.results[c]["hout"] for c in range(NCORES)], axis=0)
    return out.astype(np.float32)
